# revision 1
# baseline (speedup 1.0000x reference)
"""Trainium2 Bass kernel for nn_Attention_86431921864842.

Decode-style attention: B=16 batches, H=16 heads, Sq=16 new tokens,
4096-token KV cache, RoPE-extended 128-dim scores, fused QKV + output
projections.

Sharding: tensor-parallel over heads, 8 cores x 2 heads each.  Each core
receives the full x (bf16 hi/lo), its 2-head slice of w_qkv (transposed,
bf16 hi/lo), its 2-head column slice of w_o (transposed), and its heads'
K/rot/V caches in device-friendly layouts:

  k2hl [32, 128, 2, 4096] bf16 - per (head_local, batch): rows 0:64 =
      cache_k^T, rows 64:128 = cache_pos_k_rot^T (d on partitions);
      index 0 = bf16 hi half, 1 = bf16 lo half (hi + lo == f32 value to
      ~2^-16 relative).  Scores use hi*hi' + hi*lo' + lo*hi', giving
      f32-grade results at bf16 matmul rates.
  vv [32, 128, 32, 65] f32 - V cache tiled [p=128, n=32 tiles, 64 dims]
      plus a baked-in ones column (col 64) so the PV matmul also produces
      the softmax denominator.

Device per (b,h): S^T tiles via 2 matmuls per 128-token tile
(kh x [q_hi|q_lo] with N=32, plus kl x q_hi accumulated) + one DVE fold,
-> exp -> PV accumulate (attn^T stationary, [V|1] moving) -> per-query
normalize -> o-proj partial.  PV for pair i is emitted after S^T for
pair i+1 (one-stage software pipeline) so the Tensor engine never
stalls on the exp.  Host sums the 8 partial o-proj outputs.
"""

import math
import os
import sys

import numpy as np

for _p in ("/opt/trn_rl_repo",):
    if _p not in sys.path and os.path.isdir(_p):
        sys.path.insert(0, _p)

B = 16
H = 16
SQ = 16
DM = 1024
DH = 64
SKV = 4096
ROPE_BASE = 10000.0
N_CORES = 8
H_PER_CORE = H // N_CORES  # 2
E_PER_CORE = H_PER_CORE * 3 * DH  # 384
D_PER_CORE = H_PER_CORE * DH  # 128
BS = B * SQ  # 256
N_KTILES = SKV // 128  # 32
SCALE = 1.0 / math.sqrt(2 * DH)

_PROGRAM = None  # (nc, in_names, out_name)


def _build_program():
    import concourse.bass as bass
    import concourse.mybir as mybir
    import concourse.tile as tile
    from concourse import bacc

    f32 = mybir.dt.float32
    bf16 = mybir.dt.bfloat16
    Exp = mybir.ActivationFunctionType.Exp

    nc = bacc.Bacc(
        "TRN2",
        target_bir_lowering=False,
        debug=False,
        enable_asserts=False,
        num_devices=N_CORES,
    )

    xh_d = nc.dram_tensor("xTh", [128, 8, BS], bf16, kind="ExternalInput")
    xl_d = nc.dram_tensor("xTl", [128, 8, BS], bf16, kind="ExternalInput")
    wqhl_d = nc.dram_tensor(
        "wqhl", [128, 8, 2 * E_PER_CORE], bf16, kind="ExternalInput"
    )
    wo_d = nc.dram_tensor("woT", [D_PER_CORE, DM], f32, kind="ExternalInput")
    k2hl_d = nc.dram_tensor("k2hl", [2 * B, 128, 2, SKV], bf16, kind="ExternalInput")
    vv_d = nc.dram_tensor("vv", [2 * B, 128, N_KTILES, 65], f32, kind="ExternalInput")
    cos_d = nc.dram_tensor("cosN", [128, 32], f32, kind="ExternalInput")
    sin_d = nc.dram_tensor("sinN", [128, 32], f32, kind="ExternalInput")
    id_d = nc.dram_tensor("ident", [128, 128], f32, kind="ExternalInput")
    out_d = nc.dram_tensor("out", [2, 128, DM], f32, kind="ExternalOutput")

    with tile.TileContext(nc) as tc:
        with (
            tc.tile_pool(name="const", bufs=1) as pc,
            tc.tile_pool(name="head", bufs=1) as ph,
            tc.tile_pool(name="rope", bufs=1) as pr,
            tc.tile_pool(name="k2", bufs=4) as pk,
            tc.tile_pool(name="vc", bufs=5) as pv,
            tc.tile_pool(name="exp", bufs=2) as pe,
            tc.tile_pool(name="small", bufs=2) as ps,
            tc.tile_pool(name="ps_s", bufs=2, space="PSUM") as pss,
            tc.tile_pool(name="ps_o", bufs=2, space="PSUM") as pso,
            tc.tile_pool(name="ps_m", bufs=2, space="PSUM") as psm,
        ):
            # ---- constants ----
            xh_sb = pc.tile([128, 8, BS], bf16, tag="xh")
            nc.sync.dma_start(xh_sb[:], xh_d[:])
            xl_sb = pc.tile([128, 8, BS], bf16, tag="xl")
            nc.sync.dma_start(xl_sb[:], xl_d[:])
            wq_sb = pc.tile([128, 8, 2 * E_PER_CORE], bf16, tag="wq")
            nc.sync.dma_start(wq_sb[:], wqhl_d[:])
            cos_sb = pc.tile([128, 32], f32, tag="cos")
            nc.sync.dma_start(cos_sb[:], cos_d[:])
            sin_sb = pc.tile([128, 32], f32, tag="sin")
            nc.sync.dma_start(sin_sb[:], sin_d[:])
            id_sb = pc.tile([128, 128], f32, tag="ident")
            nc.sync.dma_start(id_sb[:], id_d[:])
            wo_sb = pc.tile([128, DM], f32, tag="wo")
            nc.sync.dma_start(wo_sb[:], wo_d[:])

            # ---- QKV projection (bf16 3-term): qkv_nat[bs, e_local] ----
            qkv_nat = ph.tile([128, 2, E_PER_CORE], f32, tag="qkv_nat")
            for j in range(2):
                psq = pss.tile([128, 1024], f32, tag="sT", name=f"psq{j}")
                for dc in range(8):
                    xh_c = xh_sb[:, dc, j * 128 : (j + 1) * 128]
                    xl_c = xl_sb[:, dc, j * 128 : (j + 1) * 128]
                    # xh*wh -> bank0 cols 0:384 ; xh*wl -> bank1 cols 512:896
                    nc.tensor.matmul(
                        psq[:, 0:E_PER_CORE],
                        lhsT=xh_c,
                        rhs=wq_sb[:, dc, 0:E_PER_CORE],
                        start=(dc == 0),
                        stop=False,
                    )
                    nc.tensor.matmul(
                        psq[:, 512 : 512 + E_PER_CORE],
                        lhsT=xh_c,
                        rhs=wq_sb[:, dc, E_PER_CORE : 2 * E_PER_CORE],
                        start=(dc == 0),
                        stop=(dc == 7),
                    )
                    nc.tensor.matmul(
                        psq[:, 0:E_PER_CORE],
                        lhsT=xl_c,
                        rhs=wq_sb[:, dc, 0:E_PER_CORE],
                        start=False,
                        stop=(dc == 7),
                    )
                nc.vector.tensor_copy(qkv_nat[:, j, :], psq[:, 0:E_PER_CORE])
                nc.vector.tensor_add(
                    qkv_nat[:, j, :],
                    qkv_nat[:, j, :],
                    psq[:, 512 : 512 + E_PER_CORE],
                )

            # ---- RoPE + transposes per local head ----
            cosb = cos_sb[:].unsqueeze(1).to_broadcast([128, 2, 32])
            sinb = sin_sb[:].unsqueeze(1).to_broadcast([128, 2, 32])
            q2T = []  # per head: [128, 256] f32 (d2, bs)
            q2HL = []  # per head: [128, 16, 32] bf16 ([q_hi | q_lo] per batch)
            k2nT = []  # per head: [128, 256] f32
            vTh = []  # per head: [64, 256] f32 (dv, bs)
            for hl in range(2):
                base = hl * 3 * DH
                qs = qkv_nat[:, :, base : base + 64]
                ks = qkv_nat[:, :, base + 64 : base + 128]

                q2n = pr.tile([128, 2, 128], f32, tag="q2n")
                k2n = pr.tile([128, 2, 128], f32, tag="k2n")
                t1 = pr.tile([128, 2, 32], f32, tag="t1")
                t2 = pr.tile([128, 2, 32], f32, tag="t2")
                for src, dst in ((qs, q2n), (ks, k2n)):
                    x1 = src[:, :, 0:32]
                    x2 = src[:, :, 32:64]
                    nc.vector.tensor_copy(dst[:, :, 0:64], src)
                    nc.vector.tensor_mul(t1[:], x1, cosb)
                    nc.vector.tensor_mul(t2[:], x2, sinb)
                    nc.vector.tensor_sub(dst[:, :, 64:96], t1[:], t2[:])
                    nc.vector.tensor_mul(t1[:], x1, sinb)
                    nc.vector.tensor_mul(t2[:], x2, cosb)
                    nc.vector.tensor_add(dst[:, :, 96:128], t1[:], t2[:])

                q2T_h = ph.tile([128, BS], f32, tag=f"q2T_{hl}")
                k2nT_h = ph.tile([128, BS], f32, tag=f"k2nT_{hl}")
                vT_h = ph.tile([64, BS], f32, tag=f"vT_{hl}")
                for j in range(2):
                    pt = psm.tile([128, 512], f32, tag="misc")
                    nc.tensor.transpose(pt[:, 0:128], q2n[:, j, :], id_sb[:])
                    nc.vector.tensor_copy(q2T_h[:, j * 128 : (j + 1) * 128], pt[:, 0:128])
                    pt2 = psm.tile([128, 512], f32, tag="misc")
                    nc.tensor.transpose(pt2[:, 0:128], k2n[:, j, :], id_sb[:])
                    nc.vector.tensor_copy(
                        k2nT_h[:, j * 128 : (j + 1) * 128], pt2[:, 0:128]
                    )
                    pt3 = psm.tile([128, 512], f32, tag="misc")
                    nc.tensor.transpose(
                        pt3[0:64, 0:128],
                        qkv_nat[:, j, base + 128 : base + 192],
                        id_sb[:],
                    )
                    nc.vector.tensor_copy(vT_h[:, j * 128 : (j + 1) * 128], pt3[0:64, 0:128])

                # bf16 hi/lo split of q2T, packed [q_hi | q_lo] per batch
                q2hl = ph.tile([128, B, 32], bf16, tag=f"q2hl_{hl}")
                q2v = q2T_h[:].rearrange("p (b q) -> p b q", q=16)
                q2w = pr.tile([128, B, 16], f32, tag="q2w")
                nc.vector.tensor_copy(q2hl[:, :, 0:16], q2v)  # round to bf16
                nc.vector.tensor_copy(q2w[:], q2hl[:, :, 0:16])  # widen back
                nc.vector.tensor_sub(q2w[:], q2v, q2w[:])  # residual
                nc.vector.tensor_copy(q2hl[:, :, 16:32], q2w[:])  # round residual

                q2T.append(q2T_h)
                q2HL.append(q2hl)
                k2nT.append(k2nT_h)
                vTh.append(vT_h)

            # ---- new-token V rows, pre-transposed to [s, (hl,b), 65] ----
            vn_all = ph.tile([16, 2, B, 65], f32, tag="vn_all")
            nc.vector.memset(vn_all[:, :, :, 64:65], 1.0)
            for hl in range(2):
                for b in range(B):
                    pvn = psm.tile([128, 512], f32, tag="misc")
                    nc.tensor.transpose(
                        pvn[0:16, 0:64],
                        vTh[hl][:, b * 16 : (b + 1) * 16],
                        id_sb[0:64, 0:64],
                    )
                    nc.vector.tensor_copy(vn_all[:, hl, b, 0:64], pvn[0:16, 0:64])

            # val_sb[s, b, hl, dv] : normalized attention output (natural)
            val_sb = ph.tile([16, B, 2, 64], f32, tag="val_sb")

            # ---- main loop over (head_local, batch), PV pipelined 1 back ----
            def emit_pv(state):
                hl, b, expT, vt = state
                ps_o = pso.tile([16, 65], f32, tag="o")
                for i in range(N_KTILES):
                    nc.tensor.matmul(
                        ps_o[:],
                        lhsT=expT[:, i * 16 : (i + 1) * 16],
                        rhs=vt[:, i, :],
                        start=(i == 0),
                        stop=False,
                    )
                nc.tensor.matmul(
                    ps_o[:],
                    lhsT=expT[0:16, 512:528],
                    rhs=vn_all[:, hl, b, :],
                    start=False,
                    stop=True,
                )
                rec = ps.tile([16, 1], f32, tag="rec")
                nc.vector.reciprocal(rec[:], ps_o[:, 64:65])
                nc.vector.tensor_mul(
                    val_sb[:, b, hl, :],
                    ps_o[:, 0:64],
                    rec[:, 0:1].to_broadcast([16, 64]),
                )

            # epilogue piece for one bs-chunk (8 batches x both heads)
            valT = ph.tile([128, 2, 128], f32, tag="valT")
            out_sb = ph.tile([128, 2, DM], f32, tag="out_sb")

            def emit_chunk_epilogue(j):
                pvt = psm.tile([128, 512], f32, tag="misc", name=f"pvt{j}")
                for bb in range(8):
                    b = j * 8 + bb
                    nc.tensor.transpose(
                        pvt[:, bb * 16 : (bb + 1) * 16],
                        val_sb[:, b, :, :],
                        id_sb[0:16, 0:16],
                    )
                nc.vector.tensor_copy(valT[:, j, :], pvt[:, 0:128])
                for h2 in range(2):
                    po = psm.tile([128, 512], f32, tag="misc", name=f"po{j}{h2}")
                    nc.tensor.matmul(
                        po[:],
                        lhsT=valT[:, j, :],
                        rhs=wo_sb[:, h2 * 512 : (h2 + 1) * 512],
                        start=True,
                        stop=True,
                    )
                    nc.vector.tensor_copy(out_sb[:, j, h2 * 512 : (h2 + 1) * 512], po[:])
                nc.sync.dma_start(out_d[j], out_sb[:, j, :])

            pending = None
            n_pv_done = 0
            for b in range(B):
                for hl in range(2):
                    bh = hl * B + b
                    k2_t = pk.tile([128, 2, SKV], bf16, tag="k2")
                    nc.sync.dma_start(k2_t[:], k2hl_d[bh])
                    vt = pv.tile([128, N_KTILES, 65], f32, tag="vt")
                    nc.scalar.dma_start(vt[:], vv_d[bh])

                    qhl = q2HL[hl][:, b, :]  # [128, 32] bf16
                    qh = q2HL[hl][:, b, 0:16]  # [128, 16] bf16

                    # new-token scores (fp32, tiny; independent of the k2
                    # DMA, so it gives PE work at the bh boundary)
                    psn = psm.tile([16, 16], f32, tag="misc", name=f"psn{bh}")
                    nc.tensor.matmul(
                        psn[:],
                        lhsT=k2nT[hl][:, b * 16 : (b + 1) * 16],
                        rhs=q2T[hl][:, b * 16 : (b + 1) * 16],
                        start=True,
                        stop=True,
                    )

                    # S^T: per 128-token tile, 2 matmuls into [32i, 32i+32):
                    #   A: kh x [qh|ql] (start+stop), B: kl x qh accumulated
                    #   onto the even half.  DVE then folds odd into even.
                    ps_sT = pss.tile([128, 1024], f32, tag="sT")
                    for i in range(N_KTILES):
                        kh = k2_t[:, 0, i * 128 : (i + 1) * 128]
                        kl = k2_t[:, 1, i * 128 : (i + 1) * 128]
                        nc.tensor.matmul(
                            ps_sT[:, i * 32 : i * 32 + 32],
                            lhsT=kh,
                            rhs=qhl,
                            start=True,
                            stop=True,
                        )
                        nc.tensor.matmul(
                            ps_sT[:, i * 32 : i * 32 + 16],
                            lhsT=kl,
                            rhs=qh,
                            start=False,
                            stop=True,
                            skip_group_check=True,
                        )
                    psv = ps_sT[:].rearrange("p (n c) -> p n c", c=32)

                    # exp(even + odd) as exp(even) * exp(odd): a DVE op may
                    # read only one PSUM operand, so fold after the exps.
                    expT = pe.tile([128, 528], f32, tag="expT")
                    etmp = pe.tile([128, 512], f32, tag="etmp")
                    ev = expT[:, 0:512].rearrange("p (n c) -> p n c", c=16)
                    et = etmp[:].rearrange("p (n c) -> p n c", c=16)
                    nc.scalar.activation(ev, psv[:, :, 0:16], Exp, scale=SCALE)
                    nc.scalar.activation(et, psv[:, :, 16:32], Exp, scale=SCALE)
                    nc.vector.tensor_mul(
                        expT[:, 0:512], expT[:, 0:512], etmp[:]
                    )
                    nc.scalar.activation(
                        expT[0:16, 512:528], psn[:], Exp, scale=SCALE
                    )

                    if pending is not None:
                        emit_pv(pending)
                        n_pv_done += 1
                        if n_pv_done == 17:
                            # batches 0..7 (both heads) fully normalized:
                            # run the first output-chunk epilogue now
                            emit_chunk_epilogue(0)
                    pending = (hl, b, expT, vt)
            emit_pv(pending)
            emit_chunk_epilogue(1)


    nc.compile()
    in_names = ["xTh", "xTl", "wqhl", "woT", "k2hl", "vv", "cosN", "sinN", "ident"]
    return nc, in_names, "out"


def _get_program():
    global _PROGRAM
    if _PROGRAM is None:
        _PROGRAM = _build_program()
    return _PROGRAM


def _prep_inputs(x, w_qkv, w_o, cache_k, cache_v, cache_pos_k_rot):
    """Host-side sharding + layout prep. Returns list of per-core in_maps."""
    import ml_dtypes

    f32 = np.float32
    bf16 = ml_dtypes.bfloat16
    x = np.ascontiguousarray(x, dtype=f32)
    w_qkv = np.ascontiguousarray(w_qkv, dtype=f32)
    w_o = np.ascontiguousarray(w_o, dtype=f32)

    xT = np.ascontiguousarray(x.reshape(BS, DM).T)
    xTh = xT.astype(bf16)
    xTl = (xT - xTh.astype(f32)).astype(bf16)
    # pre-tile to [p=128, dc=8, bs] so the const DMA is contiguous per row
    xTh = np.ascontiguousarray(xTh.reshape(8, 128, BS).transpose(1, 0, 2))
    xTl = np.ascontiguousarray(xTl.reshape(8, 128, BS).transpose(1, 0, 2))

    wqkvT = np.ascontiguousarray(w_qkv.T)  # [DM, 3*DM]
    wqh = wqkvT.astype(bf16)
    wql = (wqkvT - wqh.astype(f32)).astype(bf16)

    # k2 staging: [core, hl, b, 128, SKV] f32, then bf16 hi/lo interleave
    k2t = np.empty((N_CORES, 2, B, 128, SKV), dtype=f32)
    k2t[:, :, :, 0:64, :] = cache_k.reshape(B, N_CORES, 2, SKV, DH).transpose(
        1, 2, 0, 4, 3
    )
    k2t[:, :, :, 64:128, :] = cache_pos_k_rot.reshape(
        B, N_CORES, 2, SKV, DH
    ).transpose(1, 2, 0, 4, 3)
    k2hl = np.empty((N_CORES, 2, B, 128, 2, SKV), dtype=bf16)
    k2hl[:, :, :, :, 0, :] = k2t.astype(bf16)
    k2hl[:, :, :, :, 1, :] = (k2t - k2hl[:, :, :, :, 0, :].astype(f32)).astype(bf16)
    del k2t

    # v staging: [core, hl, b, p, n, 65]
    vv = np.empty((N_CORES, 2, B, 128, N_KTILES, 65), dtype=f32)
    vv[..., 0:64] = cache_v.reshape(B, N_CORES, 2, N_KTILES, 128, DH).transpose(
        1, 2, 0, 4, 3, 5
    )
    vv[..., 64] = 1.0

    # RoPE tables, f32 math mirroring the reference
    j2 = np.arange(0, DH, 2, dtype=f32)
    inv_freq = (1.0 / (ROPE_BASE ** (j2 / f32(DH)))).astype(f32)
    pos = (SKV + np.arange(SQ)).astype(f32)
    ang = pos[:, None] * inv_freq[None, :]  # [16, 32]
    cosN = np.tile(np.cos(ang).astype(f32), (8, 1))  # [128, 32]
    sinN = np.tile(np.sin(ang).astype(f32), (8, 1))

    ident = np.eye(128, dtype=f32)

    in_maps = []
    for c in range(N_CORES):
        wq_hl = np.concatenate(
            [
                wqh[:, c * E_PER_CORE : (c + 1) * E_PER_CORE],
                wql[:, c * E_PER_CORE : (c + 1) * E_PER_CORE],
            ],
            axis=1,
        )
        wq_hl = wq_hl.reshape(8, 128, 2 * E_PER_CORE).transpose(1, 0, 2)
        in_maps.append(
            {
                "xTh": xTh,
                "xTl": xTl,
                "wqhl": np.ascontiguousarray(wq_hl),
                "woT": np.ascontiguousarray(
                    w_o[:, c * D_PER_CORE : (c + 1) * D_PER_CORE].T
                ),
                "k2hl": k2hl[c].reshape(2 * B, 128, 2, SKV),
                "vv": vv[c].reshape(2 * B, 128, N_KTILES, 65),
                "cosN": cosN,
                "sinN": sinN,
                "ident": ident,
            }
        )
    return in_maps


def _run(in_maps, trace=False, trace_kwargs=None):
    from concourse import bass_utils

    nc, in_names, out_name = _get_program()
    kwargs = {}
    if trace:
        kwargs["trace"] = True
        if trace_kwargs:
            kwargs.update(trace_kwargs)
    res = bass_utils.run_bass_kernel_spmd(
        nc, in_maps, core_ids=list(range(N_CORES)), **kwargs
    )
    return res


def kernel(x, w_qkv, w_o, cache_k, cache_v, cache_pos_k_rot, mask=None, **_ignored):
    """Full-input entry point: shards internally across 8 NeuronCores."""
    in_maps = _prep_inputs(x, w_qkv, w_o, cache_k, cache_v, cache_pos_k_rot)
    res = _run(in_maps)
    out = np.zeros((BS, DM), dtype=np.float32)
    for c in range(N_CORES):
        out += res.results[c]["out"].reshape(BS, DM)
    return out.reshape(B, SQ, DM)



# revision 12
# speedup vs baseline: 1.8397x; 1.8397x over previous
"""Trainium2 Bass kernel for nn_Attention_86431921864842.

Decode-style attention: B=16 batches, H=16 heads, Sq=16 new tokens,
4096-token KV cache, RoPE-extended 128-dim scores, fused QKV + output
projections.

Sharding: tensor-parallel over heads, 8 cores x 2 heads each.  Each core
receives the full x (bf16 hi/lo), its 2-head slice of w_qkv (transposed,
bf16 hi/lo), its 2-head column slice of w_o (transposed), and its heads'
K/rot/V caches in device-friendly layouts:

  k2h [32, 128, 4096] bf16 - per (head_local, batch): rows 0:64 =
      cache_k^T, rows 64:128 = cache_pos_k_rot^T (d on partitions),
      rounded to bf16.  q stays hi/lo so the only score error is the
      K-cache rounding (~1e-3 on scaled scores).
  vv [32, 128, 32, 65] bf16 - V cache tiled [p=128, n=32 tiles, 64 dims]
      plus a baked-in ones column (col 64) so the PV matmul also produces
      the softmax denominator.

Device per (b,h): S^T tiles via 1 matmul per 128-token tile
(kh x [q_hi|q_lo] with N=32) + exp(hi)*exp(lo) DVE fold -> bf16 attn
-> PV accumulate (attn^T stationary, [V|1] moving, all bf16) ->
per-query normalize -> o-proj partial (bf16).  PV for pair i is emitted
after S^T for pair i+1 (one-stage software pipeline) so the Tensor
engine never stalls on the exp.  Host sums the 8 partial o-proj
outputs.
"""

import math
import os
import sys

import numpy as np

for _p in ("/opt/trn_rl_repo",):
    if _p not in sys.path and os.path.isdir(_p):
        sys.path.insert(0, _p)

B = 16
H = 16
SQ = 16
DM = 1024
DH = 64
SKV = 4096
ROPE_BASE = 10000.0
N_CORES = 8
H_PER_CORE = H // N_CORES  # 2
E_PER_CORE = H_PER_CORE * 3 * DH  # 384
D_PER_CORE = H_PER_CORE * DH  # 128
BS = B * SQ  # 256
N_KTILES = SKV // 128  # 32
SCALE = 1.0 / math.sqrt(2 * DH)

_PROGRAM = None  # (nc, in_names, out_name)


def _build_program():
    import concourse.bass as bass
    import concourse.mybir as mybir
    import concourse.tile as tile
    from concourse import bacc

    f32 = mybir.dt.float32
    bf16 = mybir.dt.bfloat16
    Exp = mybir.ActivationFunctionType.Exp

    nc = bacc.Bacc(
        "TRN2",
        target_bir_lowering=False,
        debug=False,
        enable_asserts=False,
        num_devices=N_CORES,
    )

    xh_d = nc.dram_tensor("xTh", [128, 8, BS], bf16, kind="ExternalInput")
    xl_d = nc.dram_tensor("xTl", [128, 8, BS], bf16, kind="ExternalInput")
    wqhl_d = nc.dram_tensor(
        "wqhl", [128, 8, 2 * E_PER_CORE], bf16, kind="ExternalInput"
    )
    wo_d = nc.dram_tensor("woT", [D_PER_CORE, DM], bf16, kind="ExternalInput")
    k2h_d = nc.dram_tensor("k2h", [2 * B, 128, SKV], bf16, kind="ExternalInput")
    vv_d = nc.dram_tensor("vv", [2 * B, 128, N_KTILES, 65], bf16, kind="ExternalInput")
    cos_d = nc.dram_tensor("cosN", [128, 32], f32, kind="ExternalInput")
    sin_d = nc.dram_tensor("sinN", [128, 32], f32, kind="ExternalInput")
    id_d = nc.dram_tensor("ident", [128, 128], f32, kind="ExternalInput")
    out_d = nc.dram_tensor("out", [2, 128, DM], f32, kind="ExternalOutput")

    with tile.TileContext(nc) as tc:
        with (
            tc.tile_pool(name="const", bufs=1) as pc,
            tc.tile_pool(name="head", bufs=1) as ph,
            tc.tile_pool(name="rope", bufs=1) as pr,
            tc.tile_pool(name="k2", bufs=4) as pk,
            tc.tile_pool(name="vc", bufs=5) as pv,
            tc.tile_pool(name="exp", bufs=2) as pe,
            tc.tile_pool(name="small", bufs=2) as ps,
            tc.tile_pool(name="ps_s", bufs=2, space="PSUM") as pss,
            tc.tile_pool(name="ps_o", bufs=2, space="PSUM") as pso,
            tc.tile_pool(name="ps_m", bufs=2, space="PSUM") as psm,
        ):
            # ---- constants ----
            xh_sb = pc.tile([128, 8, BS], bf16, tag="xh")
            nc.sync.dma_start(xh_sb[:], xh_d[:])
            xl_sb = pc.tile([128, 8, BS], bf16, tag="xl")
            nc.sync.dma_start(xl_sb[:], xl_d[:])
            wq_sb = pc.tile([128, 8, 2 * E_PER_CORE], bf16, tag="wq")
            nc.sync.dma_start(wq_sb[:], wqhl_d[:])
            cos_sb = pc.tile([128, 32], f32, tag="cos")
            nc.sync.dma_start(cos_sb[:], cos_d[:])
            sin_sb = pc.tile([128, 32], f32, tag="sin")
            nc.sync.dma_start(sin_sb[:], sin_d[:])
            id_sb = pc.tile([128, 128], f32, tag="ident")
            nc.sync.dma_start(id_sb[:], id_d[:])
            wo_sb = pc.tile([128, DM], bf16, tag="wo")
            nc.sync.dma_start(wo_sb[:], wo_d[:])

            # ---- QKV projection (bf16 3-term): qkv_nat[bs, e_local] ----
            qkv_nat = ph.tile([128, 2, E_PER_CORE], f32, tag="qkv_nat")
            for j in range(2):
                psq = pss.tile([128, 1024], f32, tag="sT", name=f"psq{j}")
                for dc in range(8):
                    xh_c = xh_sb[:, dc, j * 128 : (j + 1) * 128]
                    xl_c = xl_sb[:, dc, j * 128 : (j + 1) * 128]
                    # xh*wh -> bank0 cols 0:384 ; xh*wl -> bank1 cols 512:896
                    nc.tensor.matmul(
                        psq[:, 0:E_PER_CORE],
                        lhsT=xh_c,
                        rhs=wq_sb[:, dc, 0:E_PER_CORE],
                        start=(dc == 0),
                        stop=False,
                    )
                    nc.tensor.matmul(
                        psq[:, 512 : 512 + E_PER_CORE],
                        lhsT=xh_c,
                        rhs=wq_sb[:, dc, E_PER_CORE : 2 * E_PER_CORE],
                        start=(dc == 0),
                        stop=(dc == 7),
                    )
                    nc.tensor.matmul(
                        psq[:, 0:E_PER_CORE],
                        lhsT=xl_c,
                        rhs=wq_sb[:, dc, 0:E_PER_CORE],
                        start=False,
                        stop=(dc == 7),
                    )
                nc.vector.tensor_copy(qkv_nat[:, j, :], psq[:, 0:E_PER_CORE])
                nc.vector.tensor_add(
                    qkv_nat[:, j, :],
                    qkv_nat[:, j, :],
                    psq[:, 512 : 512 + E_PER_CORE],
                )

            # ---- RoPE + transposes per local head ----
            cosb = cos_sb[:].unsqueeze(1).to_broadcast([128, 2, 32])
            sinb = sin_sb[:].unsqueeze(1).to_broadcast([128, 2, 32])
            q2T = []  # per head: [128, 256] f32 (d2, bs)
            q2HL = []  # per head: [128, 16, 32] bf16 ([q_hi | q_lo] per batch)
            k2nT = []  # per head: [128, 256] f32
            vTh = []  # per head: [64, 256] f32 (dv, bs)
            for hl in range(2):
                base = hl * 3 * DH
                qs = qkv_nat[:, :, base : base + 64]
                ks = qkv_nat[:, :, base + 64 : base + 128]

                q2n = pr.tile([128, 2, 128], f32, tag="q2n")
                k2n = pr.tile([128, 2, 128], f32, tag="k2n")
                t1 = pr.tile([128, 2, 32], f32, tag="t1")
                t2 = pr.tile([128, 2, 32], f32, tag="t2")
                for src, dst in ((qs, q2n), (ks, k2n)):
                    x1 = src[:, :, 0:32]
                    x2 = src[:, :, 32:64]
                    nc.vector.tensor_copy(dst[:, :, 0:64], src)
                    nc.vector.tensor_mul(t1[:], x1, cosb)
                    nc.vector.tensor_mul(t2[:], x2, sinb)
                    nc.vector.tensor_sub(dst[:, :, 64:96], t1[:], t2[:])
                    nc.vector.tensor_mul(t1[:], x1, sinb)
                    nc.vector.tensor_mul(t2[:], x2, cosb)
                    nc.vector.tensor_add(dst[:, :, 96:128], t1[:], t2[:])

                q2T_h = ph.tile([128, BS], f32, tag=f"q2T_{hl}")
                k2nT_h = ph.tile([128, BS], f32, tag=f"k2nT_{hl}")
                vT_h = ph.tile([64, BS], f32, tag=f"vT_{hl}")
                for j in range(2):
                    pt = psm.tile([128, 512], f32, tag="misc")
                    nc.tensor.transpose(pt[:, 0:128], q2n[:, j, :], id_sb[:])
                    nc.vector.tensor_copy(q2T_h[:, j * 128 : (j + 1) * 128], pt[:, 0:128])
                    pt2 = psm.tile([128, 512], f32, tag="misc")
                    nc.tensor.transpose(pt2[:, 0:128], k2n[:, j, :], id_sb[:])
                    nc.vector.tensor_copy(
                        k2nT_h[:, j * 128 : (j + 1) * 128], pt2[:, 0:128]
                    )
                    pt3 = psm.tile([128, 512], f32, tag="misc")
                    nc.tensor.transpose(
                        pt3[0:64, 0:128],
                        qkv_nat[:, j, base + 128 : base + 192],
                        id_sb[:],
                    )
                    nc.vector.tensor_copy(vT_h[:, j * 128 : (j + 1) * 128], pt3[0:64, 0:128])

                # bf16 hi/lo split of q2T, packed [q_hi | q_lo] per batch
                q2hl = ph.tile([128, B, 32], bf16, tag=f"q2hl_{hl}")
                q2v = q2T_h[:].rearrange("p (b q) -> p b q", q=16)
                q2w = pr.tile([128, B, 16], f32, tag="q2w")
                nc.vector.tensor_copy(q2hl[:, :, 0:16], q2v)  # round to bf16
                nc.vector.tensor_copy(q2w[:], q2hl[:, :, 0:16])  # widen back
                nc.vector.tensor_sub(q2w[:], q2v, q2w[:])  # residual
                nc.vector.tensor_copy(q2hl[:, :, 16:32], q2w[:])  # round residual

                q2T.append(q2T_h)
                q2HL.append(q2hl)
                k2nT.append(k2nT_h)
                vTh.append(vT_h)

            # ---- new-token V rows, pre-transposed to [s, (hl,b), 65] ----
            vn_all = ph.tile([16, 2, B, 65], bf16, tag="vn_all")
            nc.vector.memset(vn_all[:, :, :, 64:65], 1.0)
            for hl in range(2):
                for b in range(B):
                    pvn = psm.tile([128, 512], f32, tag="misc")
                    nc.tensor.transpose(
                        pvn[0:16, 0:64],
                        vTh[hl][:, b * 16 : (b + 1) * 16],
                        id_sb[0:64, 0:64],
                    )
                    nc.vector.tensor_copy(vn_all[:, hl, b, 0:64], pvn[0:16, 0:64])

            # val_sb[s, b, hl, dv] : normalized attention output (natural)
            val_sb = ph.tile([16, B, 2, 64], f32, tag="val_sb")

            # ---- main loop over (head_local, batch), PV pipelined 1 back ----
            def emit_pv(state):
                hl, b, expT, vt = state
                ps_o = pso.tile([16, 65], f32, tag="o")
                for i in range(N_KTILES):
                    nc.tensor.matmul(
                        ps_o[:],
                        lhsT=expT[:, i * 16 : (i + 1) * 16],
                        rhs=vt[:, i, :],
                        start=(i == 0),
                        stop=False,
                    )
                nc.tensor.matmul(
                    ps_o[:],
                    lhsT=expT[0:16, 512:528],
                    rhs=vn_all[:, hl, b, :],
                    start=False,
                    stop=True,
                )
                rec = ps.tile([16, 1], f32, tag="rec")
                nc.vector.reciprocal(rec[:], ps_o[:, 64:65])
                nc.vector.tensor_mul(
                    val_sb[:, b, hl, :],
                    ps_o[:, 0:64],
                    rec[:, 0:1].to_broadcast([16, 64]),
                )

            # epilogue piece for one bs-chunk (8 batches x both heads)
            valT = ph.tile([128, 2, 128], bf16, tag="valT")
            out_sb = ph.tile([128, 2, DM], f32, tag="out_sb")

            def emit_chunk_epilogue(j):
                pvt = psm.tile([128, 512], f32, tag="misc", name=f"pvt{j}")
                for bb in range(8):
                    b = j * 8 + bb
                    nc.tensor.transpose(
                        pvt[:, bb * 16 : (bb + 1) * 16],
                        val_sb[:, b, :, :],
                        id_sb[0:16, 0:16],
                    )
                nc.vector.tensor_copy(valT[:, j, :], pvt[:, 0:128])
                for h2 in range(2):
                    po = psm.tile([128, 512], f32, tag="misc", name=f"po{j}{h2}")
                    nc.tensor.matmul(
                        po[:],
                        lhsT=valT[:, j, :],
                        rhs=wo_sb[:, h2 * 512 : (h2 + 1) * 512],
                        start=True,
                        stop=True,
                    )
                    nc.vector.tensor_copy(out_sb[:, j, h2 * 512 : (h2 + 1) * 512], po[:])
                nc.sync.dma_start(out_d[j], out_sb[:, j, :])

            pending = None
            n_pv_done = 0
            for b in range(B):
                for hl in range(2):
                    bh = hl * B + b
                    k2_t = pk.tile([128, SKV], bf16, tag="k2")
                    nc.sync.dma_start(k2_t[:], k2h_d[bh])
                    vt = pv.tile([128, N_KTILES, 65], bf16, tag="vt")
                    nc.scalar.dma_start(vt[:], vv_d[bh])

                    qhl = q2HL[hl][:, b, :]  # [128, 32] bf16

                    # new-token scores (fp32, tiny; independent of the k2
                    # DMA, so it gives PE work at the bh boundary)
                    psn = psm.tile([16, 16], f32, tag="misc", name=f"psn{bh}")
                    nc.tensor.matmul(
                        psn[:],
                        lhsT=k2nT[hl][:, b * 16 : (b + 1) * 16],
                        rhs=q2T[hl][:, b * 16 : (b + 1) * 16],
                        start=True,
                        stop=True,
                    )

                    # S^T: per 128-token tile, 1 matmul into [32i, 32i+32):
                    #   kh x [qh|ql] -> even half kh*qh, odd half kh*ql.
                    ps_sT = pss.tile([128, 1024], f32, tag="sT")
                    for i in range(N_KTILES):
                        kh = k2_t[:, i * 128 : (i + 1) * 128]
                        nc.tensor.matmul(
                            ps_sT[:, i * 32 : i * 32 + 32],
                            lhsT=kh,
                            rhs=qhl,
                            start=True,
                            stop=True,
                        )
                    psv = ps_sT[:].rearrange("p (n c) -> p n c", c=32)

                    # exp(even + odd) as exp(even) * exp(odd): a DVE op may
                    # read only one PSUM operand, so fold after the exps.
                    expT = pe.tile([128, 528], bf16, tag="expT")
                    e1 = pe.tile([128, 512], f32, tag="e1")
                    e2 = pe.tile([128, 512], f32, tag="e2")
                    ev = e1[:].rearrange("p (n c) -> p n c", c=16)
                    et = e2[:].rearrange("p (n c) -> p n c", c=16)
                    nc.scalar.activation(ev, psv[:, :, 0:16], Exp, scale=SCALE)
                    nc.scalar.activation(et, psv[:, :, 16:32], Exp, scale=SCALE)
                    nc.vector.tensor_mul(expT[:, 0:512], e1[:], e2[:])
                    nc.scalar.activation(
                        expT[0:16, 512:528], psn[:], Exp, scale=SCALE
                    )

                    if pending is not None:
                        emit_pv(pending)
                        n_pv_done += 1
                        if n_pv_done == 17:
                            # batches 0..7 (both heads) fully normalized:
                            # run the first output-chunk epilogue now
                            emit_chunk_epilogue(0)
                    pending = (hl, b, expT, vt)
            emit_pv(pending)
            emit_chunk_epilogue(1)


    nc.compile()
    in_names = ["xTh", "xTl", "wqhl", "woT", "k2h", "vv", "cosN", "sinN", "ident"]
    return nc, in_names, "out"


def _get_program():
    global _PROGRAM
    if _PROGRAM is None:
        _PROGRAM = _build_program()
    return _PROGRAM


def _prep_inputs(x, w_qkv, w_o, cache_k, cache_v, cache_pos_k_rot):
    """Host-side sharding + layout prep. Returns list of per-core in_maps."""
    import ml_dtypes

    f32 = np.float32
    bf16 = ml_dtypes.bfloat16
    x = np.ascontiguousarray(x, dtype=f32)
    w_qkv = np.ascontiguousarray(w_qkv, dtype=f32)
    w_o = np.ascontiguousarray(w_o, dtype=f32)

    xT = np.ascontiguousarray(x.reshape(BS, DM).T)
    xTh = xT.astype(bf16)
    xTl = (xT - xTh.astype(f32)).astype(bf16)
    # pre-tile to [p=128, dc=8, bs] so the const DMA is contiguous per row
    xTh = np.ascontiguousarray(xTh.reshape(8, 128, BS).transpose(1, 0, 2))
    xTl = np.ascontiguousarray(xTl.reshape(8, 128, BS).transpose(1, 0, 2))

    wqkvT = np.ascontiguousarray(w_qkv.T)  # [DM, 3*DM]
    wqh = wqkvT.astype(bf16)
    wql = (wqkvT - wqh.astype(f32)).astype(bf16)

    # k2 staging: [core, hl, b, 128, SKV] bf16 (rounded)
    k2h = np.empty((N_CORES, 2, B, 128, SKV), dtype=bf16)
    k2h[:, :, :, 0:64, :] = (
        cache_k.reshape(B, N_CORES, 2, SKV, DH)
        .transpose(1, 2, 0, 4, 3)
        .astype(bf16)
    )
    k2h[:, :, :, 64:128, :] = (
        cache_pos_k_rot.reshape(B, N_CORES, 2, SKV, DH)
        .transpose(1, 2, 0, 4, 3)
        .astype(bf16)
    )

    # v staging: [core, hl, b, p, n, 65] bf16
    vv = np.empty((N_CORES, 2, B, 128, N_KTILES, 65), dtype=bf16)
    vv[..., 0:64] = (
        cache_v.reshape(B, N_CORES, 2, N_KTILES, 128, DH)
        .transpose(1, 2, 0, 4, 3, 5)
        .astype(bf16)
    )
    vv[..., 64] = 1.0

    # RoPE tables, f32 math mirroring the reference
    j2 = np.arange(0, DH, 2, dtype=f32)
    inv_freq = (1.0 / (ROPE_BASE ** (j2 / f32(DH)))).astype(f32)
    pos = (SKV + np.arange(SQ)).astype(f32)
    ang = pos[:, None] * inv_freq[None, :]  # [16, 32]
    cosN = np.tile(np.cos(ang).astype(f32), (8, 1))  # [128, 32]
    sinN = np.tile(np.sin(ang).astype(f32), (8, 1))

    ident = np.eye(128, dtype=f32)

    in_maps = []
    for c in range(N_CORES):
        wq_hl = np.concatenate(
            [
                wqh[:, c * E_PER_CORE : (c + 1) * E_PER_CORE],
                wql[:, c * E_PER_CORE : (c + 1) * E_PER_CORE],
            ],
            axis=1,
        )
        wq_hl = wq_hl.reshape(8, 128, 2 * E_PER_CORE).transpose(1, 0, 2)
        in_maps.append(
            {
                "xTh": xTh,
                "xTl": xTl,
                "wqhl": np.ascontiguousarray(wq_hl),
                "woT": np.ascontiguousarray(
                    w_o[:, c * D_PER_CORE : (c + 1) * D_PER_CORE].T
                ).astype(bf16),
                "k2h": k2h[c].reshape(2 * B, 128, SKV),
                "vv": vv[c].reshape(2 * B, 128, N_KTILES, 65),
                "cosN": cosN,
                "sinN": sinN,
                "ident": ident,
            }
        )
    return in_maps


def _run(in_maps, trace=False, trace_kwargs=None):
    from concourse import bass_utils

    nc, in_names, out_name = _get_program()
    kwargs = {}
    if trace:
        kwargs["trace"] = True
        if trace_kwargs:
            kwargs.update(trace_kwargs)
    res = bass_utils.run_bass_kernel_spmd(
        nc, in_maps, core_ids=list(range(N_CORES)), **kwargs
    )
    return res


def kernel(x, w_qkv, w_o, cache_k, cache_v, cache_pos_k_rot, mask=None, **_ignored):
    """Full-input entry point: shards internally across 8 NeuronCores."""
    in_maps = _prep_inputs(x, w_qkv, w_o, cache_k, cache_v, cache_pos_k_rot)
    res = _run(in_maps)
    out = np.zeros((BS, DM), dtype=np.float32)
    for c in range(N_CORES):
        out += res.results[c]["out"].reshape(BS, DM)
    return out.reshape(B, SQ, DM)

